# revision 49
# baseline (speedup 1.0000x reference)
"""ChannelAwareAttentionModule TRN2 kernel.

Math (per sample s, all biases are no-ops because InstanceNorm removes them):
  thetaN/phiN/gN = relu(instnorm(w @ x))        [Ci=128, N=4096]
  f = thetaN @ phiN^T                           [128, 128]
  attn = softmax(f, axis=1)
  y = attn @ gN                                 [128, 4096]
  y_view[ci, q*128+r] = y[r, 32*ci+q]           (permute+reshape view)
  out = instnorm(W_w @ y_view) + x              [256, 4096]

Sharding: data-parallel over batch, 2 samples per core, 8 cores.

Layouts on core:
  x_sb      [128, 2, 4096]  (2 chunks of 128 input channels)
  projN     [128, 4096]     per projection, f32r, produced by fused
                            instnorm+relu eviction from PSUM
  thetaT/phiT [128, 32, 128] transposed tiles (PE transpose + evict)
  Z = y_view [128, 4096]    built directly: block q = (gN[:, q::32])^T @ attnT
  final conv + instnorm fused into PSUM eviction, residual add from x_sb
"""
import sys

sys.path.insert(0, "/opt/trn_rl_repo")

import numpy as np

import concourse.bass as bass
import concourse.bacc as bacc
import concourse.tile as tile
from concourse import mybir
from concourse.bass_utils import run_bass_kernel_spmd
from concourse.masks import make_identity

N_CORES = 8
B, C, CI, H, W = 16, 256, 128, 64, 64
N = H * W  # 4096
B_LOC = B // N_CORES  # 2 samples per core
KCH = C // 128  # 2 contraction chunks of the input channels
NT = N // 128  # 32 column tiles
FCH = N // 512  # 8 psum-bank-sized chunks
EPS = 1e-5

F32 = mybir.dt.float32
F32R = mybir.dt.float32r

_CACHE = {}


def build_nc():
    nc = bacc.Bacc("TRN2", target_bir_lowering=False)

    x_ext = nc.declare_dram_parameter("x", [B_LOC, C, N], F32R, isOutput=False)
    # stacked projection weights, host layout [128, KCH, 3, 128] = [c128, k, proj, ci]
    w_ext = nc.declare_dram_parameter("w3", [128, KCH, 3, CI], F32R, isOutput=False)
    ww_ext = nc.declare_dram_parameter("ww", [CI, C], F32R, isOutput=False)
    out_ext = nc.declare_dram_parameter("out", [B_LOC, C, N], F32, isOutput=True)

    with tile.TileContext(nc) as tc:
        from contextlib import ExitStack

        with ExitStack() as ctx:
            consts = ctx.enter_context(tc.tile_pool(name="consts", bufs=1))
            xpool = ctx.enter_context(tc.tile_pool(name="xpool", bufs=2))
            pn = ctx.enter_context(tc.tile_pool(name="pn", bufs=3))
            gn = ctx.enter_context(tc.tile_pool(name="gn", bufs=1))
            pT = ctx.enter_context(tc.tile_pool(name="pT", bufs=1))
            zpool = ctx.enter_context(tc.tile_pool(name="zpool", bufs=1))
            small = ctx.enter_context(tc.tile_pool(name="small", bufs=3))
            rawp = ctx.enter_context(tc.tile_pool(name="rawp", bufs=2))
            banks = ctx.enter_context(tc.tile_pool(name="banks", bufs=8, space="PSUM"))

            # ---- constants ----
            ident32 = consts.tile([128, 128], F32)
            make_identity(nc, ident32[:])
            ones1 = consts.tile([1, 128], F32)
            nc.vector.memset(ones1[:], 1.0)
            ones1r = consts.tile([1, 128], F32R)
            nc.vector.tensor_copy(ones1r[:], ones1[:])
            w_sb = consts.tile([128, KCH, 3, CI], F32R)
            nc.scalar.dma_start(w_sb[:], w_ext[:])
            ww_sb = consts.tile([CI, C], F32R)
            nc.scalar.dma_start(ww_sb[:], ww_ext[:])
            eps_t = consts.tile([128, 1], F32)
            nc.vector.memset(eps_t[:], EPS)

            def rstd_negmr(mv):
                """mv [128,2]=(mean,var) -> (rstd, -mean*rstd, sqrt(var+eps)).

                (Sqrt keeps ACT table swaps at ~5/run; exp+ln variants
                measured worse -- the compiler thrashes between tables.)"""
                sq = small.tile([128, 1], F32, tag="sq")
                nc.scalar.activation(
                    sq[:], mv[:, 1:2], mybir.ActivationFunctionType.Sqrt,
                    bias=eps_t[:], scale=1.0,
                )
                rstd = small.tile([128, 1], F32, tag="rstd")
                nc.vector.reciprocal(rstd[:], sq[:])
                negmr = small.tile([128, 1], F32, tag="negmr")
                nc.vector.tensor_scalar(
                    negmr[:], mv[:, 0:1], rstd[:], -1.0,
                    op0=mybir.AluOpType.mult, op1=mybir.AluOpType.mult,
                )
                return rstd, negmr, sq

            # ================= per-sample stage functions =================
            state = [dict() for _ in range(B_LOC)]

            def warmup():
                """~4us of junk matmuls so the PE HAM un-throttles before the
                first real matmul (runs during the initial x DMA)."""
                junk = banks.tile([128, 512], F32, tag="bank")
                for _ in range(64):
                    nc.tensor.matmul(
                        junk[:, 0:32], ident32[:], ident32[:, 0:32],
                        start=True, stop=True,
                    )

            def load_x(s):
                x_sb = xpool.tile([128, KCH, N], F32R, tag="x")
                edges = [0, 512, 2048, 4096]
                for b in range(len(edges) - 1):
                    cols = slice(edges[b], edges[b + 1])
                    for k in range(KCH):
                        nc.sync.dma_start(
                            x_sb[:, k, cols],
                            x_ext[s, 128 * k:128 * (k + 1), cols],
                        )
                state[s]["x"] = x_sb

            NRAW = 4

            def proj(s, p):
                """One projection with fused instnorm+relu eviction.

                The last NRAW chunks are copied out of PSUM raw (no rstd
                dependency) so those banks free early and later-emitted PE
                work keeps flowing through the stats window."""
                x_sb = state[s]["x"]
                ps_list = []
                stats = small.tile([128, FCH, 6], F32, tag="stats")
                rawbuf = rawp.tile([128, NRAW, 512], F32, tag="rawbuf")
                for fc in range(FCH):
                    ps = banks.tile([128, 512], F32, tag="bank")
                    for k in range(KCH):
                        nc.tensor.matmul(
                            ps[:], w_sb[:, k, p, :],
                            x_sb[:, k, 512 * fc:512 * (fc + 1)],
                            start=(k == 0), stop=(k == KCH - 1),
                        )
                    nc.vector.bn_stats(stats[:, fc, :], ps[:])
                    if fc >= FCH - NRAW:
                        dst = rawbuf[:, fc - (FCH - NRAW), :]
                        if fc % 2 == 0:
                            nc.scalar.copy(dst, ps[:])
                        else:
                            nc.vector.tensor_copy(dst, ps[:])
                    ps_list.append(ps)
                mv = small.tile([128, 2], F32, tag="mv")
                nc.vector.bn_aggr(mv[:], stats[:])
                rstd, negmr, _ = rstd_negmr(mv)
                projN = (gn if p == 2 else pn).tile(
                    [128, N], F32R if p == 2 else F32,
                    tag=("g" if p == 2 else "pn"))
                for fc in range(FCH):
                    dst = projN[:, 512 * fc:512 * (fc + 1)]
                    if fc < FCH - NRAW:
                        nc.scalar.activation(
                            dst, ps_list[fc][:],
                            mybir.ActivationFunctionType.Relu,
                            bias=negmr[:], scale=rstd[:],
                        )
                    else:
                        src_ap = rawbuf[:, fc - (FCH - NRAW), :]
                        nc.scalar.activation(
                            dst, src_ap,
                            mybir.ActivationFunctionType.Relu,
                            bias=negmr[:], scale=rstd[:],
                        )
                state[s]["ptg"[p]] = projN

            def transposes(s):
                # TP[:, t, 0, :] = phiT_t, TP[:, t, 1, :] = thetaT_t so the
                # gram rhs [phiT|thetaT] is one contiguous [128,256] slice
                # (256-wide moving dim = full-rate f32r).
                TP = pT.tile([128, NT, 2, 128], F32R, tag="pT")
                nev = 0
                for slot, psrc in ((1, state[s]["p"]), (0, state[s]["t"])):
                    for t4 in range(NT // 4):
                        tp = banks.tile([128, 4, 128], F32, tag="bank")
                        for j in range(4):
                            t = 4 * t4 + j
                            nc.tensor.transpose(
                                tp[:, j, :],
                                psrc[:, 128 * t:128 * (t + 1)],
                                ident32[:],
                            )
                        dst4 = TP[:, 4 * t4:4 * (t4 + 1), slot, :]
                        if nev % 3 == 2:
                            nc.scalar.copy(dst4, tp[:])
                        else:
                            nc.vector.tensor_copy(dst4, tp[:])
                        nev += 1
                state[s]["TP"] = TP

            def gram_softmax(s):
                # wait: slot 1 = thetaT (lhsT), [phi|theta] rhs -> f in cols 0:128
                TP = state[s]["TP"]
                f_ps = banks.tile([128, 256], F32, tag="bank")
                for t in range(NT):
                    nc.tensor.matmul(
                        f_ps[:], TP[:, t, 1, :], TP[:, t, :, :],
                        start=(t == 0), stop=(t == NT - 1),
                    )
                f_ap = f_ps[:, 0:128]
                negmax = small.tile([128, 1], F32, tag="negmax")
                nc.vector.tensor_reduce(
                    negmax[:], f_ap, axis=mybir.AxisListType.X,
                    op=mybir.AluOpType.max, negate=True,
                )
                attn_e = small.tile([128, 128], F32, tag="attn_e")
                sumexp = small.tile([128, 1], F32, tag="sumexp")
                nc.scalar.activation(
                    attn_e[:], f_ap, mybir.ActivationFunctionType.Exp,
                    bias=negmax[:], scale=1.0, accum_out=sumexp[:],
                )
                rsum = small.tile([128, 1], F32, tag="rsum")
                nc.vector.reciprocal(rsum[:], sumexp[:])
                attn_n = small.tile([128, 128], F32, tag="attn_n")
                nc.vector.tensor_scalar_mul(attn_n[:], attn_e[:], rsum[:])
                state[s]["attn_n"] = attn_n

            def attn_transpose(s):
                at_ps = banks.tile([128, 128], F32, tag="bank")
                nc.tensor.transpose(at_ps[:], state[s]["attn_n"][:], ident32[:])
                at_ps = at_ps[:]
                # duplicated [attnT|attnT] so Z matmuls run 256-wide
                attnT = small.tile([128, 2, 128], F32R, tag="attnT")
                nc.vector.tensor_copy(attnT[:, 0, :], at_ps)
                nc.vector.tensor_copy(attnT[:, 1, :], at_ps)
                state[s]["attnT"] = attnT

            def zstage(s):
                gN = state[s]["g"]
                attnT = state[s]["attnT"]
                Z = zpool.tile([128, N], F32R, tag="z")
                g3 = gN[:].rearrange("p (c q) -> p c q", q=NT)
                for q2 in range(NT // 2):
                    zp = banks.tile([128, 2, 256], F32, tag="bank")
                    for j in range(2):
                        q = 2 * q2 + j
                        nc.tensor.matmul(
                            zp[:, j, :], g3[:, :, q],
                            attnT[:].rearrange("p a b -> p (a b)"),
                            start=True, stop=True,
                        )
                    zdst = Z[:, 256 * q2:256 * (q2 + 1)].rearrange(
                        "p (j c) -> p j c", j=2)
                    if q2 % 3 == 2:
                        nc.scalar.copy(zdst, zp[:, :, 0:128])
                    else:
                        nc.vector.tensor_copy(zdst, zp[:, :, 0:128])
                state[s]["Z"] = Z

            def conv_stats(s, oc):
                """Stats-only pass over W@Z: banks recycle immediately."""
                Z = state[s]["Z"]
                stats2 = small.tile([128, FCH, 6], F32, tag="stats2")
                for fc in range(FCH):
                    ps = banks.tile([128, 512], F32, tag="bank")
                    nc.tensor.matmul(
                        ps[:], ww_sb[:, 128 * oc:128 * (oc + 1)],
                        Z[:, 512 * fc:512 * (fc + 1)],
                        start=True, stop=True,
                    )
                    nc.vector.bn_stats(stats2[:, fc, :], ps[:])
                mv2 = small.tile([128, 2], F32, tag="mv2")
                nc.vector.bn_aggr(mv2[:], stats2[:])
                rstd2, negmr2, sq2 = rstd_negmr(mv2)
                diag = small.tile([128, 128], F32R, tag="diag")
                nc.vector.tensor_scalar_mul(diag[:], ident32[:], sq2[:])
                state[s].setdefault("cstats", []).append((rstd2, negmr2, diag))

            def conv_out(s):
                """Recompute W@Z chunk, accumulate x*sqrt(var+eps) via diag
                matmul, evict rstd*psum+negmr = instnorm(W@Z)+x, DMA out.
                oc=1 writes normalize output in-place over Z (its chunks are
                no longer needed once the last oc=1 matmul has read them)."""
                Z = state[s]["Z"]
                x_sb = state[s]["x"]
                normed0 = pn.tile([128, N], F32, tag="pn")
                order = [(oc, fc) for oc in range(KCH) for fc in range(FCH)]
                for oc, fc in order:
                    if True:
                        rstd2, negmr2, diag = state[s]["cstats"][oc]
                        cols = slice(512 * fc, 512 * (fc + 1))
                        ps = banks.tile([128, 512], F32, tag="bank")
                        nc.tensor.matmul(
                            ps[:], ww_sb[:, 128 * oc:128 * (oc + 1)],
                            Z[:, cols], start=True, stop=False,
                        )
                        nc.tensor.matmul(
                            ps[:], diag[:], x_sb[:, oc, cols],
                            start=False, stop=True,
                        )
                        dst = normed0[:, cols] if oc == 0 else Z[:, cols]
                        if fc % 4 != 3:
                            nc.scalar.activation(
                                dst, ps[:],
                                mybir.ActivationFunctionType.Identity,
                                bias=negmr2[:], scale=rstd2[:],
                            )
                        else:
                            nc.vector.tensor_scalar(
                                dst, ps[:], rstd2[:], negmr2[:],
                                op0=mybir.AluOpType.mult,
                                op1=mybir.AluOpType.add,
                            )
                        if fc % 2 == 1:
                            dcols = slice(1024 * (fc // 2), 1024 * (fc // 2 + 1))
                            srcd = (normed0[:, dcols] if oc == 0
                                    else Z[:, dcols].bitcast(F32))
                            nc.sync.dma_start(
                                out_ext[s, 128 * oc:128 * (oc + 1), dcols],
                                srcd,
                            )

            # ================= pipelined emission =================
            load_x(0)
            load_x(1)
            proj(0, 0)
            proj(0, 1)
            proj(0, 2)
            transposes(0)
            proj(1, 0)
            gram_softmax(0)
            proj(1, 1)
            attn_transpose(0)
            zstage(0)
            conv_stats(0, 0)
            conv_stats(0, 1)
            proj(1, 2)
            conv_out(0)
            transposes(1)
            gram_softmax(1)
            attn_transpose(1)
            zstage(1)
            conv_stats(1, 0)
            conv_stats(1, 1)
            conv_out(1)

    nc.compile()
    return nc


def _get_nc():
    if "nc" not in _CACHE:
        _CACHE["nc"] = build_nc()
    return _CACHE["nc"]


def _prep_in_maps(x, g_w, theta_w, phi_w, W_w):
    # stacked projection lhsT: [c, ci] chunks -> [128, KCH, 3, CI]
    w3 = np.stack(
        [theta_w.T.reshape(KCH, 128, CI), phi_w.T.reshape(KCH, 128, CI),
         g_w.T.reshape(KCH, 128, CI)],
        axis=2,
    )  # [KCH, 128, 3, CI]
    w3 = np.ascontiguousarray(w3.transpose(1, 0, 2, 3), dtype=np.float32)
    ww = np.ascontiguousarray(W_w.T, dtype=np.float32)  # [CI, C]
    xr = np.ascontiguousarray(x.reshape(B, C, N), dtype=np.float32)
    in_maps = []
    for c in range(N_CORES):
        in_maps.append({
            "x": xr[B_LOC * c:B_LOC * (c + 1)],
            "w3": w3,
            "ww": ww,
        })
    return in_maps


def kernel(x, g_w, g_b, theta_w, theta_b, phi_w, phi_b, W_w, W_b, **_ignored):
    # biases are mathematically dropped by the InstanceNorms
    nc = _get_nc()
    in_maps = _prep_in_maps(x, g_w, theta_w, phi_w, W_w)
    res = run_bass_kernel_spmd(nc, in_maps, core_ids=list(range(N_CORES)))
    outs = [res.results[c]["out"].reshape(B_LOC, C, H, W) for c in range(N_CORES)]
    return np.concatenate(outs, axis=0).astype(np.float32)


def _install_ntff_hook():
    """Provide antenv.axon_hooks if the image lacks it (see trn_boot.py)."""
    import types
    try:
        from antenv.axon_hooks import get_axon_ntff_profile_hook  # noqa: F401
        return
    except ImportError:
        pass
    import contextlib
    import ctypes

    so_path = "/opt/axon/libaxon_pjrt.so"
    lib = ctypes.CDLL(so_path)
    if not hasattr(lib, "axon_start_nrt_profile"):
        hook = None
    else:
        lib.axon_start_nrt_profile.argtypes = [
            ctypes.POINTER(ctypes.c_int64), ctypes.c_size_t]
        lib.axon_start_nrt_profile.restype = ctypes.c_int64
        lib.axon_stop_nrt_profile.argtypes = [ctypes.c_char_p]
        lib.axon_stop_nrt_profile.restype = ctypes.c_int64

        @contextlib.contextmanager
        def hook(output_dir, device_ids):
            import jax
            jax.devices()
            if device_ids:
                ids = (ctypes.c_int64 * len(device_ids))(*device_ids)
                rc = lib.axon_start_nrt_profile(ids, len(device_ids))
            else:
                rc = lib.axon_start_nrt_profile(None, 0)
            if rc != 0:
                raise RuntimeError(f"axon_start_nrt_profile rc={rc}")
            try:
                yield
            finally:
                n = lib.axon_stop_nrt_profile(str(output_dir).encode())
                if n <= 0:
                    raise RuntimeError(f"axon_stop_nrt_profile rc={n}")

    mod = types.ModuleType("antenv.axon_hooks")
    mod.get_axon_ntff_profile_hook = lambda: hook
    mod.set_axon_ntff_profile_hook = lambda h: None
    sys.modules["antenv.axon_hooks"] = mod


def run_traced(x, g_w, g_b, theta_w, theta_b, phi_w, phi_b, W_w, W_b, **_ignored):
    """Like kernel() but with NTFF profiling; returns (out, BassKernelResults)."""
    _install_ntff_hook()
    nc = _get_nc()
    in_maps = _prep_in_maps(x, g_w, theta_w, phi_w, W_w)
    res = run_bass_kernel_spmd(
        nc, in_maps, core_ids=list(range(N_CORES)), trace=True
    )
    outs = [res.results[c]["out"].reshape(B_LOC, C, H, W) for c in range(N_CORES)]
    return np.concatenate(outs, axis=0).astype(np.float32), res
